# revision 1
# baseline (speedup 1.0000x reference)
"""Trainium2 Bass kernel for EnergyForcesModelBase (gnn_message_passing).

Strategy (8 NeuronCores, SPMD, one NEFF):
  - Energy head: shard atoms by molecule range (125 mols/core; batch_idx is
    sorted, so each core gets a contiguous atom range). Per core:
    e = h_energy @ W_energy via PE matvec tiles (emb=256 -> 2 K-halves,
    host-transposed layout), then scatter-sum into 125 mols via a one-hot
    matmul (is_equal against an iota row).
  - Forces head: sort edges by target atom on host, shard by contiguous
    atom range (12500 atoms/core) so scatters are core-local. Per core:
    s = h_forces @ W_forces via PE matvec (host-transposed tiles),
    f = (s + b) * V_st (DVE per-partition scalar mult), scatter-add into
    a PSUM-resident [128 x 98*3] force accumulator via one-hot matmuls.
    Edges are grouped into 128-atom blocks; a shared (max-over-cores)
    tiles-per-block schedule keeps the instruction stream identical on
    all cores (SPMD), with zero-padded slots (V=0 kills pad contributions).
Outputs come back in packed [128, X] layouts and are unsharded on host.
"""

import numpy as np

import concourse.bass as bass
import concourse.mybir as mybir
import concourse.tile as tile
from concourse import bacc
from concourse.bass_utils import run_bass_kernel_spmd

P = 128
N_ATOMS = 100000
N_EDGES = 625000
N_MOL = 1000
EMB_ATOM = 256
EMB_EDGE = 128
NCORES = 8

ATOMS_PC = N_ATOMS // NCORES          # 12500 atoms per core
MOLS_PC = N_MOL // NCORES             # 125 mols per core
NBLK = (ATOMS_PC + P - 1) // P        # 98 atom blocks per core
F32 = mybir.dt.float32

FCHUNK = 16                           # force tiles per DMA chunk (16*128 edges)
ECHUNK = 16                           # energy tiles per DMA chunk


def _build_nc(t_force, blk_of_tile, t_energy):
    """Build the SPMD Bass program (identical for all cores)."""
    nc = bacc.Bacc(target_bir_lowering=False, debug=False)

    s_f = t_force * P                 # force edge slots
    s_e = t_energy * P                # energy atom slots

    hfT = nc.dram_tensor("hfT", [P, s_f], F32, kind="ExternalInput")
    vp = nc.dram_tensor("vp", [P, 3 * t_force], F32, kind="ExternalInput")
    ri = nc.dram_tensor("ri", [P, t_force], F32, kind="ExternalInput")
    heT0 = nc.dram_tensor("heT0", [P, s_e], F32, kind="ExternalInput")
    heT1 = nc.dram_tensor("heT1", [P, s_e], F32, kind="ExternalInput")
    rm = nc.dram_tensor("rm", [P, t_energy], F32, kind="ExternalInput")
    wf = nc.dram_tensor("wf", [P, 1], F32, kind="ExternalInput")
    we = nc.dram_tensor("we", [P, 2], F32, kind="ExternalInput")
    bb = nc.dram_tensor("bb", [P, 1], F32, kind="ExternalInput")
    iota = nc.dram_tensor("iota", [P, P], F32, kind="ExternalInput")
    f_out = nc.dram_tensor("F_out", [P, NBLK * 3], F32, kind="ExternalOutput")
    y_out = nc.dram_tensor("y_out", [P, 1], F32, kind="ExternalOutput")

    with tile.TileContext(nc) as tc:
        with (
            tc.tile_pool(name="const", bufs=1) as cpool,
            tc.tile_pool(name="hfs", bufs=3) as hfpool,
            tc.tile_pool(name="hes", bufs=3) as hepool,
            tc.tile_pool(name="work", bufs=4) as wpool,
            tc.tile_pool(name="sps", bufs=2, space="PSUM") as spspool,
            tc.tile_pool(name="acc", bufs=1, space="PSUM") as accpool,
        ):
            # Resident constants
            v_sb = cpool.tile([P, 3 * t_force], F32)
            ri_sb = cpool.tile([P, t_force], F32)
            rm_sb = cpool.tile([P, t_energy], F32)
            wf_sb = cpool.tile([P, 1], F32)
            we_sb = cpool.tile([P, 2], F32)
            b_sb = cpool.tile([P, 1], F32)
            io_sb = cpool.tile([P, P], F32)
            for dst, src in [(v_sb, vp), (ri_sb, ri), (rm_sb, rm), (wf_sb, wf),
                             (we_sb, we), (b_sb, bb), (io_sb, iota)]:
                nc.sync.dma_start(out=dst[:], in_=src[:])

            # Persistent PSUM accumulators
            f_psum = accpool.tile([P, NBLK * 3], F32)
            y_psum = accpool.tile([P, 1], F32)

            # ---------------- forces head ----------------
            for c0 in range(0, t_force, FCHUNK):
                c1 = min(c0 + FCHUNK, t_force)
                n_t = c1 - c0
                hf_sb = hfpool.tile([P, FCHUNK * P], F32, tag="hf")
                nc.sync.dma_start(out=hf_sb[:, :n_t * P],
                                  in_=hfT[:, c0 * P:c1 * P])
                s_ps = spspool.tile([P, FCHUNK], F32, tag="sps")
                for i in range(n_t):
                    nc.tensor.matmul(
                        out=s_ps[:, i:i + 1],
                        lhsT=hf_sb[:, i * P:(i + 1) * P],
                        rhs=wf_sb[:],
                        start=True, stop=True,
                    )
                s_sb = wpool.tile([P, FCHUNK], F32, tag="ssb")
                nc.vector.tensor_scalar(
                    out=s_sb[:, :n_t], in0=s_ps[:, :n_t],
                    scalar1=b_sb[:, 0:1], scalar2=None,
                    op0=mybir.AluOpType.add,
                )
                for i in range(n_t):
                    t = c0 + i
                    fe = wpool.tile([P, 3], F32, tag="fe")
                    nc.vector.tensor_scalar(
                        out=fe[:], in0=v_sb[:, 3 * t:3 * t + 3],
                        scalar1=s_sb[:, i:i + 1], scalar2=None,
                        op0=mybir.AluOpType.mult,
                    )
                    oh = wpool.tile([P, P], F32, tag="oh")
                    nc.vector.tensor_scalar(
                        out=oh[:], in0=io_sb[:],
                        scalar1=ri_sb[:, t:t + 1], scalar2=None,
                        op0=mybir.AluOpType.is_equal,
                    )
                    b = blk_of_tile[t]
                    first = t == 0 or blk_of_tile[t - 1] != b
                    last = t == t_force - 1 or blk_of_tile[t + 1] != b
                    nc.tensor.matmul(
                        out=f_psum[:, 3 * b:3 * b + 3],
                        lhsT=oh[:], rhs=fe[:],
                        start=first, stop=last,
                    )
            f_sb = cpool.tile([P, NBLK * 3], F32)
            nc.scalar.copy(out=f_sb[:], in_=f_psum[:])
            nc.sync.dma_start(out=f_out[:], in_=f_sb[:])

            # ---------------- energy head ----------------
            for c0 in range(0, t_energy, ECHUNK):
                c1 = min(c0 + ECHUNK, t_energy)
                n_t = c1 - c0
                he0_sb = hepool.tile([P, ECHUNK * P], F32, tag="he0")
                he1_sb = hepool.tile([P, ECHUNK * P], F32, tag="he1")
                nc.sync.dma_start(out=he0_sb[:, :n_t * P],
                                  in_=heT0[:, c0 * P:c1 * P])
                nc.sync.dma_start(out=he1_sb[:, :n_t * P],
                                  in_=heT1[:, c0 * P:c1 * P])
                e_ps = spspool.tile([P, ECHUNK], F32, tag="eps")
                for i in range(n_t):
                    nc.tensor.matmul(
                        out=e_ps[:, i:i + 1],
                        lhsT=he0_sb[:, i * P:(i + 1) * P],
                        rhs=we_sb[:, 0:1],
                        start=True, stop=False,
                    )
                    nc.tensor.matmul(
                        out=e_ps[:, i:i + 1],
                        lhsT=he1_sb[:, i * P:(i + 1) * P],
                        rhs=we_sb[:, 1:2],
                        start=False, stop=True,
                    )
                e_sb = wpool.tile([P, ECHUNK], F32, tag="esb")
                nc.scalar.copy(out=e_sb[:, :n_t], in_=e_ps[:, :n_t])
                for i in range(n_t):
                    t = c0 + i
                    ohm = wpool.tile([P, P], F32, tag="ohm")
                    nc.vector.tensor_scalar(
                        out=ohm[:], in0=io_sb[:],
                        scalar1=rm_sb[:, t:t + 1], scalar2=None,
                        op0=mybir.AluOpType.is_equal,
                    )
                    nc.tensor.matmul(
                        out=y_psum[:],
                        lhsT=ohm[:], rhs=e_sb[:, i:i + 1],
                        start=(t == 0), stop=(t == t_energy - 1),
                    )
            y_sb = cpool.tile([P, 1], F32)
            nc.scalar.copy(out=y_sb[:], in_=y_psum[:])
            nc.sync.dma_start(out=y_out[:], in_=y_sb[:])

    nc.finalize()
    return nc


def _prep_inputs(h_energy, h_forces, V_st, idx_t, batch_idx,
                 W_energy, W_forces, b_forces):
    """Host-side sharding/packing. Returns (in_maps, t_force, blk_of_tile,
    t_energy, meta for unsharding)."""
    idx_t = np.asarray(idx_t)
    batch_idx = np.asarray(batch_idx)
    h_energy = np.asarray(h_energy, dtype=np.float32)
    h_forces = np.asarray(h_forces, dtype=np.float32)
    V_st = np.asarray(V_st, dtype=np.float32)

    # ---- forces: sort edges by target atom, shard by atom range ----
    perm = np.argsort(idx_t, kind="stable")
    idx_s = idx_t[perm]
    e_bound = np.searchsorted(idx_s, np.arange(0, N_ATOMS + 1, ATOMS_PC))
    # per-core per-block counts -> shared schedule
    counts = np.zeros((NCORES, NBLK), dtype=np.int64)
    for c in range(NCORES):
        loc = idx_s[e_bound[c]:e_bound[c + 1]] - c * ATOMS_PC
        counts[c] = np.bincount(loc // P, minlength=NBLK)
    tpb = np.maximum(1, -(-counts.max(axis=0) // P))       # tiles per block
    t_force = int(tpb.sum())
    blk_of_tile = np.repeat(np.arange(NBLK), tpb)
    blk_slot0 = np.concatenate([[0], np.cumsum(tpb)])[:-1] * P

    # ---- energy: shard atoms by molecule range ----
    a_bound = np.searchsorted(batch_idx, np.arange(0, N_MOL + 1, MOLS_PC))
    n_at = np.diff(a_bound)
    t_energy = int(-(-n_at.max() // P))
    s_e = t_energy * P

    iota = np.tile(np.arange(P, dtype=np.float32), (P, 1))
    b_val = np.float32(np.asarray(b_forces).reshape(-1)[0])

    in_maps = []
    for c in range(NCORES):
        # forces packing
        eids = perm[e_bound[c]:e_bound[c + 1]]
        loc = idx_s[e_bound[c]:e_bound[c + 1]] - c * ATOMS_PC
        blk = loc // P
        # slot index: edges are sorted by atom hence by block; offset within
        # block = position among this core's edges of the same block
        within = np.arange(len(loc)) - np.concatenate(
            [[0], np.cumsum(counts[c])])[:-1][blk]
        slots = blk_slot0[blk] + within
        s_f = t_force * P
        buf = np.zeros((s_f, EMB_EDGE), dtype=np.float32)
        buf[slots] = h_forces[eids]
        hfT = np.ascontiguousarray(buf.T)
        vbuf = np.zeros((s_f, 3), dtype=np.float32)
        vbuf[slots] = V_st[eids]
        vp = np.ascontiguousarray(
            vbuf.reshape(t_force, P, 3).transpose(1, 0, 2).reshape(P, -1))
        ribuf = np.zeros(s_f, dtype=np.float32)
        ribuf[slots] = (loc - P * blk).astype(np.float32)
        ri = np.ascontiguousarray(ribuf.reshape(t_force, P).T)

        # energy packing
        a0, a1 = a_bound[c], a_bound[c + 1]
        ebuf = np.zeros((s_e, EMB_ATOM), dtype=np.float32)
        ebuf[:a1 - a0] = h_energy[a0:a1]
        heT = np.ascontiguousarray(ebuf.T)
        rmbuf = np.zeros(s_e, dtype=np.float32)
        rmbuf[:a1 - a0] = (batch_idx[a0:a1] - c * MOLS_PC).astype(np.float32)
        rmp = np.ascontiguousarray(rmbuf.reshape(t_energy, P).T)

        in_maps.append({
            "hfT": hfT,
            "vp": vp,
            "ri": ri,
            "heT0": np.ascontiguousarray(heT[:P]),
            "heT1": np.ascontiguousarray(heT[P:]),
            "rm": rmp,
            "wf": np.asarray(W_forces, dtype=np.float32).reshape(P, 1),
            "we": np.ascontiguousarray(
                np.asarray(W_energy, dtype=np.float32).reshape(2, P).T),
            "bb": np.full((P, 1), b_val, dtype=np.float32),
            "iota": iota,
        })
    meta = (a_bound,)
    return in_maps, t_force, blk_of_tile, t_energy, meta


def _unshard(results, meta):
    (a_bound,) = meta
    force = np.empty((N_ATOMS, 3), dtype=np.float32)
    y = np.empty((N_MOL,), dtype=np.float32)
    for c in range(NCORES):
        fp = results[c]["F_out"]                     # [P, NBLK*3]
        blkmat = fp.reshape(P, NBLK, 3).transpose(1, 0, 2).reshape(NBLK * P, 3)
        force[c * ATOMS_PC:(c + 1) * ATOMS_PC] = blkmat[:ATOMS_PC]
        y[c * MOLS_PC:(c + 1) * MOLS_PC] = results[c]["y_out"][:MOLS_PC, 0]
    return y, force


def kernel_with_stats(trace=False, **inputs):
    in_maps, t_force, blk_of_tile, t_energy, meta = _prep_inputs(**inputs)
    nc = _build_nc(t_force, blk_of_tile, t_energy)
    res = run_bass_kernel_spmd(nc, in_maps, core_ids=list(range(NCORES)),
                               trace=trace)
    y, force = _unshard(res.results, meta)
    return (y, force), res


def kernel(**inputs):
    out, _ = kernel_with_stats(trace=False, **inputs)
    return out
